# revision 3
# baseline (speedup 1.0000x reference)
"""Bass/Trainium2 kernel for nn_BayesConv2dMF (per-sample-weight 3x3 conv).

Contract: kernel(**inputs) takes FULL unsharded inputs
  input      [32, 128, 56, 56] f32
  eps        [32, 128, 128, 3, 3] f32
  weight_psi [128, 128, 3, 3] f32
  weight_mu  [128, 128, 3, 3] f32
and returns the FULL output [32, 128, 56, 56] f32.

Strategy: data-parallel over batch across 8 NeuronCores (4 images/core).
Per image on-core:
  w  = eps * exp(psi) + mu                      (DVE, bf16 out)
  wT = per-tap PE transpose -> [CI, 9, CO]      (TensorE + DVE evac)
  x  -> zero-padded [CI, 58, 58] bf16           (DMA + DVE cast)
  out[CO, 8, 56] chunks = sum_k wT[k].T @ xpad[shifted window]   (9 PSUM-accum matmuls)
  PSUM -> SBUF (ScalarE) -> DRAM
"""

import numpy as np

import concourse.bass as bass
import concourse.tile as tile
from concourse import bacc, mybir
from concourse.bass_utils import run_bass_kernel_spmd
from concourse.masks import make_identity

B, CO, CI, KH, KW, H, W = 32, 128, 128, 3, 3, 56, 56
K9 = KH * KW
N_CORES = 8
BPC = B // N_CORES  # images per core
HP, WP = H + 2, W + 2  # padded image
RB = 8  # output rows per PSUM chunk
NCHUNK = H // RB
F32 = mybir.dt.float32
BF16 = mybir.dt.bfloat16


def emit(nc, tc, ctx, x_d, eps_d, psi_d, mu_d, out_d):
    const = ctx.enter_context(tc.tile_pool(name="const", bufs=1))
    wpool = ctx.enter_context(tc.tile_pool(name="wpool", bufs=2))
    xpool = ctx.enter_context(tc.tile_pool(name="xpool", bufs=2))
    opool = ctx.enter_context(tc.tile_pool(name="opool", bufs=2))
    psw = ctx.enter_context(tc.tile_pool(name="psw", bufs=2, space="PSUM"))
    pso = ctx.enter_context(tc.tile_pool(name="pso", bufs=3, space="PSUM"))

    ident = const.tile([128, 128], BF16)
    make_identity(nc, ident)

    psi_t = const.tile([CO, CI, K9], F32)
    nc.sync.dma_start(psi_t, psi_d.rearrange("co ci kh kw -> co ci (kh kw)"))
    exp_psi = const.tile([CO, CI, K9], F32)
    nc.scalar.activation(exp_psi, psi_t, mybir.ActivationFunctionType.Exp)
    mu_t = const.tile([CO, CI, K9], F32)
    nc.sync.dma_start(mu_t, mu_d.rearrange("co ci kh kw -> co ci (kh kw)"))

    # persistent padded-input tiles; borders stay zero across images
    xpads = []
    for i in range(2):
        xp = const.tile([CI, HP, WP], BF16, name=f"xpad{i}", tag=f"xpad{i}")
        nc.gpsimd.memset(xp[:], 0.0)
        xpads.append(xp)

    for b in range(BPC):
        # per-sample weights, natural layout [CO, CI, K9]
        eps_t = wpool.tile([CO, CI, K9], F32, tag="eps")
        nc.sync.dma_start(eps_t, eps_d[b].rearrange("co ci kh kw -> co ci (kh kw)"))
        nc.vector.tensor_mul(eps_t, eps_t, exp_psi)
        w_bf = wpool.tile([CO, CI, K9], BF16, tag="wbf")
        nc.vector.tensor_add(w_bf, eps_t, mu_t)

        # transpose each tap to [CI, CO]
        psum_wt = psw.tile([CI, K9, CO], BF16, tag="pswt")
        for k in range(K9):
            nc.tensor.transpose(psum_wt[:, k, :], w_bf[:, :, k], ident)
        wT = wpool.tile([CI, K9, CO], BF16, tag="wT")
        nc.vector.tensor_copy(wT, psum_wt)

        # input image, cast into padded bf16 tile
        x_f = xpool.tile([CI, H, W], F32, tag="xf")
        nc.sync.dma_start(x_f, x_d[b])
        xp = xpads[b % 2]
        nc.vector.tensor_copy(xp[:, 1 : H + 1, 1 : W + 1], x_f)

        out_sb = opool.tile([CO, H, W], F32, tag="osb")
        for c in range(NCHUNK):
            ps = pso.tile([CO, RB, W], F32, tag="pso")
            for k in range(K9):
                kh, kw = divmod(k, KW)
                nc.tensor.matmul(
                    ps,
                    wT[:, k, :],
                    xp[:, c * RB + kh : c * RB + kh + RB, kw : kw + W],
                    start=(k == 0),
                    stop=(k == K9 - 1),
                )
            nc.scalar.copy(out_sb[:, c * RB : (c + 1) * RB, :], ps)
        nc.sync.dma_start(out_d[b], out_sb)


def build():
    from contextlib import ExitStack

    nc = bacc.Bacc("TRN2", target_bir_lowering=False, debug=False, num_devices=N_CORES)
    x_d = nc.dram_tensor("input", [BPC, CI, H, W], F32, kind="ExternalInput").ap()
    eps_d = nc.dram_tensor(
        "eps", [BPC, CO, CI, KH, KW], F32, kind="ExternalInput"
    ).ap()
    psi_d = nc.dram_tensor(
        "weight_psi", [CO, CI, KH, KW], F32, kind="ExternalInput"
    ).ap()
    mu_d = nc.dram_tensor("weight_mu", [CO, CI, KH, KW], F32, kind="ExternalInput").ap()
    out_d = nc.dram_tensor("out", [BPC, CO, H, W], F32, kind="ExternalOutput").ap()

    with tile.TileContext(nc) as tc:
        with ExitStack() as ctx:
            emit(nc, tc, ctx, x_d, eps_d, psi_d, mu_d, out_d)
    nc.compile()
    return nc


_NC_CACHE = None


def kernel(input, eps, weight_psi, weight_mu, **run_kwargs):
    global _NC_CACHE
    if _NC_CACHE is None:
        _NC_CACHE = build()
    nc = _NC_CACHE
    in_maps = []
    for c in range(N_CORES):
        sl = slice(c * BPC, (c + 1) * BPC)
        in_maps.append(
            {
                "input": np.ascontiguousarray(input[sl], dtype=np.float32),
                "eps": np.ascontiguousarray(eps[sl], dtype=np.float32),
                "weight_psi": np.ascontiguousarray(weight_psi, dtype=np.float32),
                "weight_mu": np.ascontiguousarray(weight_mu, dtype=np.float32),
            }
        )
    res = run_bass_kernel_spmd(
        nc, in_maps, core_ids=list(range(N_CORES)), **run_kwargs
    )
    out = np.concatenate([res.results[c]["out"] for c in range(N_CORES)], axis=0)
    kernel._last_results = res
    return out


# revision 24
# speedup vs baseline: 1.1105x; 1.1105x over previous
"""Bass/Trainium2 kernel for nn_BayesConv2dMF (per-sample-weight 3x3 conv).

Contract: kernel(**inputs) takes FULL unsharded inputs
  input      [32, 128, 56, 56] f32
  eps        [32, 128, 128, 3, 3] f32
  weight_psi [128, 128, 3, 3] f32
  weight_mu  [128, 128, 3, 3] f32
and returns the FULL output [32, 128, 56, 56] f32.

Strategy: data-parallel over batch across 8 NeuronCores (4 images/core).
Per image on-core:
  w  = eps * exp(psi) + mu                      (DVE, bf16 out)
  wT = per-tap PE transpose -> [CI, 9, CO]      (TensorE + DVE evac)
  x  -> zero-padded [CI, 58, 58] bf16           (DMA + DVE cast)
  out[CO, 8, 56] chunks = sum_k wT[k].T @ xpad[shifted window]   (9 PSUM-accum matmuls)
  PSUM -> SBUF (ScalarE) -> DRAM
"""

import numpy as np

import concourse.bass as bass
import concourse.tile as tile
from concourse import bacc, mybir
from concourse.bass_utils import run_bass_kernel_spmd
from concourse.masks import make_identity

B, CO, CI, KH, KW, H, W = 32, 128, 128, 3, 3, 56, 56
K9 = KH * KW
N_CORES = 8
BPC = B // N_CORES  # images per core
HP, WP = H + 2, W + 2  # padded image
RB = 8  # output rows per PSUM chunk
NCHUNK = H // RB
F32 = mybir.dt.float32
BF16 = mybir.dt.bfloat16


def emit(nc, tc, ctx, x_d, eps_d, psi_d, mu_d, out_d):
    const = ctx.enter_context(tc.tile_pool(name="const", bufs=1))
    wpool = ctx.enter_context(tc.tile_pool(name="wpool", bufs=2))
    opool = ctx.enter_context(tc.tile_pool(name="opool", bufs=2))
    psw = ctx.enter_context(tc.tile_pool(name="psw", bufs=1, space="PSUM"))
    pso = ctx.enter_context(tc.tile_pool(name="pso", bufs=1, space="PSUM"))

    ident = const.tile([128, 128], BF16)
    make_identity(nc, ident)
    ident_f = const.tile([128, 128], F32)
    make_identity(nc, ident_f)

    psi_t = const.tile([CO, CI, K9], F32)
    nc.sync.dma_start(psi_t, psi_d.rearrange("co ci kh kw -> co ci (kh kw)"))
    exp_psi = const.tile([CO, CI, K9], F32)
    nc.scalar.activation(exp_psi, psi_t, mybir.ActivationFunctionType.Exp)
    mu_t = const.tile([CO, CI, K9], F32)
    nc.sync.dma_start(mu_t, mu_d.rearrange("co ci kh kw -> co ci (kh kw)"))

    muT = const.tile([CI, K9, CO], F32)

    def emit_muT():
        # one-time: muT[ci, k, co] = mu[co, ci, k] via fp32 PE transposes
        # (transpose is linear, so wT = (eps*exp(psi))^T + muT)
        for g in range(3):
            psum_mut = psw.tile(
                [CI, 3, CO], F32, tag="psmut", name=f"psmut{g}", bufs=2
            )
            for j in range(3):
                k = 3 * g + j
                nc.tensor.transpose(psum_mut[:, j, :], mu_t[:, :, k], ident_f)
            nc.scalar.copy(muT[:, 3 * g : 3 * g + 3, :], psum_mut)

    # persistent padded-input tiles; borders stay zero across images
    NXP = 3
    xpads = []
    for i in range(NXP):
        xp = const.tile([CI, HP, WP], BF16, name=f"xpad{i}", tag=f"xpad{i}")
        # only the borders need zeroing (interior is overwritten every image)
        nc.vector.memset(xp[:, 0, :], 0.0)
        nc.vector.memset(xp[:, HP - 1, :], 0.0)
        nc.vector.memset(xp[:, 1 : HP - 1, 0 : WP : WP - 1], 0.0)
        xpads.append(xp)

    HALF = H // 2  # 28 output rows per half
    RB2 = 7  # rows per PSUM chunk
    NCH = HALF // RB2  # 4 chunks per half, all live in PSUM (k-outer loop)

    wTs = {}
    out_sbs = {}

    def prep(b):
        # per-sample weights: wm = eps * exp(psi) in bf16 (natural layout),
        # transpose each tap, add muT during the PSUM evacuation.
        # Image 0 takes the direct path (add natural mu before transposing)
        # so its critical chain doesn't wait for muT.
        eps_t = wpool.tile([CO, CI, K9], F32, tag="eps", name=f"eps{b}")
        nc.sync.dma_start(eps_t, eps_d[b].rearrange("co ci kh kw -> co ci (kh kw)"))
        w_bf = wpool.tile([CO, CI, K9], BF16, tag="wbf", name=f"wbf{b}")
        if b == 0:
            nc.vector.tensor_mul(eps_t, eps_t, exp_psi)
            nc.vector.tensor_add(w_bf, eps_t, mu_t)
        else:
            nc.vector.tensor_mul(w_bf, eps_t, exp_psi)

        psum_wt = psw.tile([CI, K9, CO], BF16, tag="pswt", name=f"pswt{b}")
        for k in range(K9):
            nc.tensor.transpose(psum_wt[:, k, :], w_bf[:, :, k], ident)
        wT = wpool.tile([CI, K9, CO], BF16, tag="wT", name=f"wT{b}")
        if b == 0:
            nc.scalar.copy(wT, psum_wt)
        else:
            nc.vector.tensor_add(wT, psum_wt, muT)
        wTs[b] = wT

        # input image: SWDGE cast-DMA (f32->bf16) straight into the padded
        # tile, split so taps kh<2 of the first half can start early
        xp = xpads[b % NXP]
        nc.gpsimd.dma_start(
            xp[:, 1 : HALF + 3, 1 : W + 1], x_d[b][:, 0 : HALF + 2, :]
        )
        nc.gpsimd.dma_start(
            xp[:, HALF + 3 : H + 1, 1 : W + 1], x_d[b][:, HALF + 2 : H, :]
        )
        out_sbs[b] = opool.tile([CO, H, W], F32, tag="osb", name=f"osb{b}")

    def conv_half(b, hh, last=False):
        xp = xpads[b % NXP]
        wT = wTs[b]
        out_sb = out_sbs[b]
        r0 = hh * HALF
        pss = []
        for c in range(NCH):
            ps = pso.tile(
                [CO, RB2, W], F32, tag=f"pso{c}", name=f"ps{c}", bufs=1
            )
            pss.append(ps)
        # taps outer: one weight load per tap feeds all 4 chunk matmuls
        for k in range(K9):
            kh, kw = divmod(k, KW)
            for c in range(NCH):
                rr = r0 + c * RB2 + kh
                nc.tensor.matmul(
                    pss[c],
                    wT[:, k, :],
                    xp[:, rr : rr + RB2, kw : kw + W],
                    start=(k == 0),
                    stop=(k == K9 - 1),
                )
        for c in range(NCH):
            dst = out_sb[:, r0 + c * RB2 : r0 + (c + 1) * RB2, :]
            # steady state: keep DVE free for the next image's weight path
            if last and c % 2 == 1:
                nc.vector.tensor_copy(dst, pss[c])
            else:
                nc.scalar.copy(dst, pss[c])
        if last:
            # split the final store so it overlaps the remaining evacuations
            q = HALF // 2
            nc.sync.dma_start(
                out_d[b][:, r0 : r0 + q, :], out_sb[:, r0 : r0 + q, :]
            )
            nc.sync.dma_start(
                out_d[b][:, r0 + q : r0 + HALF, :],
                out_sb[:, r0 + q : r0 + HALF, :],
            )
        else:
            nc.sync.dma_start(
                out_d[b][:, r0 : r0 + HALF, :], out_sb[:, r0 : r0 + HALF, :]
            )

    # software-pipelined emission: image b+1's weight/x prep is emitted
    # between the two conv halves of image b so its DVE/PE work interleaves
    prep(0)
    emit_muT()
    for b in range(BPC):
        conv_half(b, 0)
        if b + 1 < BPC:
            prep(b + 1)
        conv_half(b, 1, last=(b == BPC - 1))


def build():
    from contextlib import ExitStack

    nc = bacc.Bacc("TRN2", target_bir_lowering=False, debug=False, num_devices=N_CORES)
    x_d = nc.dram_tensor("input", [BPC, CI, H, W], F32, kind="ExternalInput").ap()
    eps_d = nc.dram_tensor(
        "eps", [BPC, CO, CI, KH, KW], F32, kind="ExternalInput"
    ).ap()
    psi_d = nc.dram_tensor(
        "weight_psi", [CO, CI, KH, KW], F32, kind="ExternalInput"
    ).ap()
    mu_d = nc.dram_tensor("weight_mu", [CO, CI, KH, KW], F32, kind="ExternalInput").ap()
    out_d = nc.dram_tensor("out", [BPC, CO, H, W], F32, kind="ExternalOutput").ap()

    with tile.TileContext(nc) as tc:
        with ExitStack() as ctx:
            emit(nc, tc, ctx, x_d, eps_d, psi_d, mu_d, out_d)
    nc.compile()
    return nc


_NC_CACHE = None


def kernel(input, eps, weight_psi, weight_mu, **run_kwargs):
    global _NC_CACHE
    if _NC_CACHE is None:
        _NC_CACHE = build()
    nc = _NC_CACHE
    in_maps = []
    for c in range(N_CORES):
        sl = slice(c * BPC, (c + 1) * BPC)
        in_maps.append(
            {
                "input": np.ascontiguousarray(input[sl], dtype=np.float32),
                "eps": np.ascontiguousarray(eps[sl], dtype=np.float32),
                "weight_psi": np.ascontiguousarray(weight_psi, dtype=np.float32),
                "weight_mu": np.ascontiguousarray(weight_mu, dtype=np.float32),
            }
        )
    res = run_bass_kernel_spmd(
        nc, in_maps, core_ids=list(range(N_CORES)), **run_kwargs
    )
    out = np.concatenate([res.results[c]["out"] for c in range(N_CORES)], axis=0)
    kernel._last_results = res
    return out


# revision 32
# speedup vs baseline: 1.1363x; 1.0232x over previous
"""Bass/Trainium2 kernel for nn_BayesConv2dMF (per-sample-weight 3x3 conv).

Contract: kernel(**inputs) takes FULL unsharded inputs
  input      [32, 128, 56, 56] f32
  eps        [32, 128, 128, 3, 3] f32
  weight_psi [128, 128, 3, 3] f32
  weight_mu  [128, 128, 3, 3] f32
and returns the FULL output [32, 128, 56, 56] f32.

Strategy: data-parallel over batch across 8 NeuronCores (4 images/core).
Per image on-core:
  w  = eps * exp(psi) + mu                      (DVE, bf16 out)
  wT = per-tap PE transpose -> [CI, 9, CO]      (TensorE + DVE evac)
  x  -> zero-padded [CI, 58, 58] bf16           (DMA + DVE cast)
  out[CO, 8, 56] chunks = sum_k wT[k].T @ xpad[shifted window]   (9 PSUM-accum matmuls)
  PSUM -> SBUF (ScalarE) -> DRAM
"""

import numpy as np

import concourse.bass as bass
import concourse.tile as tile
from concourse import bacc, mybir
from concourse.bass_utils import run_bass_kernel_spmd
from concourse.masks import make_identity
from concourse.tile_rust import add_dep_helper

B, CO, CI, KH, KW, H, W = 32, 128, 128, 3, 3, 56, 56
K9 = KH * KW
N_CORES = 8
BPC = B // N_CORES  # images per core
HP, WP = H + 2, W + 2  # padded image
RB = 8  # output rows per PSUM chunk
NCHUNK = H // RB
F32 = mybir.dt.float32
BF16 = mybir.dt.bfloat16


def emit(nc, tc, ctx, x_d, eps_d, psi_d, mu_d, out_d):
    const = ctx.enter_context(tc.tile_pool(name="const", bufs=1))
    wpool = ctx.enter_context(tc.tile_pool(name="wpool", bufs=2))
    opool = ctx.enter_context(tc.tile_pool(name="opool", bufs=2))
    psw = ctx.enter_context(tc.tile_pool(name="psw", bufs=1, space="PSUM"))
    pso = ctx.enter_context(tc.tile_pool(name="pso", bufs=1, space="PSUM"))

    ident = const.tile([128, 128], BF16)
    make_identity(nc, ident)
    ident_f = const.tile([128, 128], F32)
    make_identity(nc, ident_f)

    psi_t = const.tile([CO, CI, K9], F32)
    nc.sync.dma_start(psi_t, psi_d.rearrange("co ci kh kw -> co ci (kh kw)"))
    exp_psi = const.tile([CO, CI, K9], F32)
    nc.scalar.activation(exp_psi, psi_t, mybir.ActivationFunctionType.Exp)
    mu_t = const.tile([CO, CI, K9], F32)
    nc.sync.dma_start(mu_t, mu_d.rearrange("co ci kh kw -> co ci (kh kw)"))

    muT = const.tile([CI, K9, CO], F32)

    def emit_muT():
        # one-time: muT[ci, k, co] = mu[co, ci, k] via fp32 PE transposes
        # (transpose is linear, so wT = (eps*exp(psi))^T + muT)
        for g in range(3):
            psum_mut = psw.tile(
                [CI, 3, CO], F32, tag="psmut", name=f"psmut{g}", bufs=2
            )
            for j in range(3):
                k = 3 * g + j
                nc.tensor.transpose(psum_mut[:, j, :], mu_t[:, :, k], ident_f)
            nc.scalar.copy(muT[:, 3 * g : 3 * g + 3, :], psum_mut)

    # persistent padded-input tiles; borders stay zero across images
    NXP = 3
    xpads = []
    for i in range(NXP):
        xp = const.tile([CI, HP, WP], BF16, name=f"xpad{i}", tag=f"xpad{i}")
        # only the borders need zeroing (interior is overwritten every image)
        nc.vector.memset(xp[:, 0, :], 0.0)
        nc.vector.memset(xp[:, HP - 1, :], 0.0)
        nc.vector.memset(xp[:, 1 : HP - 1, 0 : WP : WP - 1], 0.0)
        xpads.append(xp)

    HALF = H // 2  # 28 output rows per half
    RB2 = 7  # rows per PSUM chunk
    NCH = HALF // RB2  # 4 chunks per half, all live in PSUM (k-outer loop)

    wTs = {}
    out_sbs = {}
    last_x_dma = {}

    def prep(b):
        # per-sample weights: wm = eps * exp(psi) in bf16 (natural layout),
        # transpose each tap, add muT during the PSUM evacuation.
        # Image 0 takes the direct path (add natural mu before transposing)
        # so its critical chain doesn't wait for muT.
        eps_t = wpool.tile([CO, CI, K9], F32, tag="eps", name=f"eps{b}")
        eps_dma = nc.sync.dma_start(
            eps_t, eps_d[b].rearrange("co ci kh kw -> co ci (kh kw)")
        )
        if b - 1 in last_x_dma:
            # keep the DMA queue in need-order: image b-1's x before eps[b]
            add_dep_helper(
                eps_dma.ins,
                last_x_dma[b - 1].ins,
                sync=False,
                reason="eps prefetch after previous image x load",
            )
        w_bf = wpool.tile([CO, CI, K9], BF16, tag="wbf", name=f"wbf{b}")
        psum_wt = psw.tile([CI, K9, CO], BF16, tag="pswt", name=f"pswt{b}")
        wT = wpool.tile([CI, K9, CO], BF16, tag="wT", name=f"wT{b}")
        if b == 0:
            # image 0 is ramp-critical: pipeline the weight chain per
            # 3-tap group (direct mu add; no muT dependency)
            for g in range(3):
                sl = slice(3 * g, 3 * g + 3)
                nc.vector.tensor_mul(
                    eps_t[:, :, sl], eps_t[:, :, sl], exp_psi[:, :, sl]
                )
                nc.vector.tensor_add(
                    w_bf[:, :, sl], eps_t[:, :, sl], mu_t[:, :, sl]
                )
                for k in range(3 * g, 3 * g + 3):
                    nc.tensor.transpose(psum_wt[:, k, :], w_bf[:, :, k], ident)
                nc.scalar.copy(wT[:, sl, :], psum_wt[:, sl, :])
        else:
            nc.vector.tensor_mul(w_bf, eps_t, exp_psi)
            for k in range(K9):
                nc.tensor.transpose(psum_wt[:, k, :], w_bf[:, :, k], ident)
            nc.vector.tensor_add(wT, psum_wt, muT)
        wTs[b] = wT

        # input image: SWDGE cast-DMA (f32->bf16) straight into the padded
        # tile, split so taps kh<2 of the first half can start early
        xp = xpads[b % NXP]
        nc.gpsimd.dma_start(
            xp[:, 1 : HALF + 3, 1 : W + 1], x_d[b][:, 0 : HALF + 2, :]
        )
        last_x_dma[b] = nc.gpsimd.dma_start(
            xp[:, HALF + 3 : H + 1, 1 : W + 1], x_d[b][:, HALF + 2 : H, :]
        )
        out_sbs[b] = opool.tile([CO, H, W], F32, tag="osb", name=f"osb{b}")

    def conv_part(b, r0, nch, pso_off, last=False):
        xp = xpads[b % NXP]
        wT = wTs[b]
        out_sb = out_sbs[b]
        rows = nch * RB2
        pss = []
        for c in range(nch):
            ps = pso.tile(
                [CO, RB2, W],
                F32,
                tag=f"pso{pso_off + c}",
                name=f"ps{pso_off + c}",
                bufs=1,
            )
            pss.append(ps)
        # taps outer: one weight load per tap feeds all live chunk matmuls
        for k in range(K9):
            kh, kw = divmod(k, KW)
            for c in range(nch):
                rr = r0 + c * RB2 + kh
                nc.tensor.matmul(
                    pss[c],
                    wT[:, k, :],
                    xp[:, rr : rr + RB2, kw : kw + W],
                    start=(k == 0),
                    stop=(k == K9 - 1),
                )
        for c in range(nch):
            dst = out_sb[:, r0 + c * RB2 : r0 + (c + 1) * RB2, :]
            # steady state: keep DVE free for the next image's weight path
            if last and c % 2 == 1:
                nc.vector.tensor_copy(dst, pss[c])
            else:
                nc.scalar.copy(dst, pss[c])
        nc.sync.dma_start(
            out_d[b][:, r0 : r0 + rows, :], out_sb[:, r0 : r0 + rows, :]
        )

    # software-pipelined emission: image b+1's weight/x prep is emitted
    # between the conv parts of image b so its DVE/PE work interleaves.
    # The final image ends with two small 2-chunk parts so the last store
    # overlaps compute and the drain tail shrinks.
    prep(0)
    emit_muT()
    for b in range(BPC):
        conv_part(b, 0, NCH, 0)
        if b + 1 < BPC:
            prep(b + 1)
            conv_part(b, HALF, NCH, 0)
        else:
            conv_part(b, HALF, 2, 0)
            conv_part(b, HALF + 2 * RB2, 2, 2, last=True)


def build():
    from contextlib import ExitStack

    nc = bacc.Bacc("TRN2", target_bir_lowering=False, debug=False, num_devices=N_CORES)
    x_d = nc.dram_tensor("input", [BPC, CI, H, W], F32, kind="ExternalInput").ap()
    eps_d = nc.dram_tensor(
        "eps", [BPC, CO, CI, KH, KW], F32, kind="ExternalInput"
    ).ap()
    psi_d = nc.dram_tensor(
        "weight_psi", [CO, CI, KH, KW], F32, kind="ExternalInput"
    ).ap()
    mu_d = nc.dram_tensor("weight_mu", [CO, CI, KH, KW], F32, kind="ExternalInput").ap()
    out_d = nc.dram_tensor("out", [BPC, CO, H, W], F32, kind="ExternalOutput").ap()

    with tile.TileContext(nc) as tc:
        with ExitStack() as ctx:
            emit(nc, tc, ctx, x_d, eps_d, psi_d, mu_d, out_d)
    nc.compile()
    return nc


_NC_CACHE = None


def kernel(input, eps, weight_psi, weight_mu, **run_kwargs):
    global _NC_CACHE
    if _NC_CACHE is None:
        _NC_CACHE = build()
    nc = _NC_CACHE
    in_maps = []
    for c in range(N_CORES):
        sl = slice(c * BPC, (c + 1) * BPC)
        in_maps.append(
            {
                "input": np.ascontiguousarray(input[sl], dtype=np.float32),
                "eps": np.ascontiguousarray(eps[sl], dtype=np.float32),
                "weight_psi": np.ascontiguousarray(weight_psi, dtype=np.float32),
                "weight_mu": np.ascontiguousarray(weight_mu, dtype=np.float32),
            }
        )
    res = run_bass_kernel_spmd(
        nc, in_maps, core_ids=list(range(N_CORES)), **run_kwargs
    )
    out = np.concatenate([res.results[c]["out"] for c in range(N_CORES)], axis=0)
    kernel._last_results = res
    return out
